# revision 1
# baseline (speedup 1.0000x reference)
"""CCPL contrastive-loss kernel for Trainium2 (8 NeuronCores).

Strategy: the loss only touches 256 sampled 3x3 neighborhoods of
feat_q/feat_k (~4.7 MB of each 512 MiB tensor), so the kernel never
streams the full tensors.  Work is data-parallel over the batch dim:
core b receives feat_q[b] / feat_k[b] (64 MiB each staged to HBM) and a
program with the 256 sample windows baked in as static strided DMAs
(sample_ids are host-known at build time, identical for every core, so
the program is SPMD-clean).  Each core gathers [64c, 256s, 9] blocks for
q and k, normalizes over the channel dim, and emits one partial
sum(|q_hat - k_hat|); the host sums the 8 partials and divides by the
element count.
"""

import os
import sys
from contextlib import ExitStack

import numpy as np

sys.path.insert(0, "/opt/trn_rl_repo")

import concourse.bass as bass
import concourse.tile as tile
from concourse import mybir
from concourse.bass_utils import run_bass_kernel_spmd


def _install_ntff_hook():
    """Provide antenv.axon_hooks when the agent image lacks it.

    concourse's axon trace path imports antenv.axon_hooks to fetch the
    NTFF profile hook; this image's antenv has no such submodule.  The
    hook implementation ships in trn_agent_boot.trn_boot, so wire it up
    against the axon PJRT .so directly.
    """
    try:
        from antenv.axon_hooks import get_axon_ntff_profile_hook  # noqa: F401

        return
    except ImportError:
        pass
    import types

    hook = None
    try:
        from trn_agent_boot.trn_boot import _ntff_profile_via_ctypes

        so = "/opt/axon/libaxon_pjrt.so"
        if os.path.exists(so):
            hook = _ntff_profile_via_ctypes(so)
    except Exception:
        hook = None
    mod = types.ModuleType("antenv.axon_hooks")
    _state = {"hook": hook}
    mod.get_axon_ntff_profile_hook = lambda: _state["hook"]
    mod.set_axon_ntff_profile_hook = lambda h: _state.update(hook=h)
    import antenv

    sys.modules["antenv.axon_hooks"] = mod
    antenv.axon_hooks = mod


_install_ntff_hook()

B, C, H, W = 8, 64, 512, 512
NUM_S = 256
EPS = 1e-7
NCOL = NUM_S * 9  # 2304 columns: (sample, 3x3 window) with center at j=4
CHUNK = 384  # matmul moving-free <= 512; 6 even chunks
NCHUNK = NCOL // CHUNK
N_CORES = 8

_cache: dict = {}
LAST_RESULTS = None  # BassKernelResults of the most recent run (for test.py)


def _split_multi_waits(nc):
    """Walrus build here embeds at most ONE sync wait per instruction.

    Tile emits instructions (notably the kernel-tail Drain) carrying many
    sem waits.  Hoist all but the last wait of any such instruction onto
    single-wait NOPs inserted immediately before it on the same queue —
    the queue stalls on each NOP in turn, preserving semantics.
    """
    from concourse import mybir as _mybir

    for f in nc.m.functions:
        for blk in f.blocks:
            insts = blk.instructions
            i = 0
            while i < len(insts):
                inst = insts[i]
                si = inst.sync_info
                if si is not None and si.on_wait and len(si.on_wait) > 1:
                    waits = list(si.on_wait)
                    si.on_wait = waits[-1:]
                    for j, w in enumerate(waits[:-1]):
                        nop = _mybir.InstNoOp(
                            name=nc.get_next_instruction_name(),
                            ins=[],
                            outs=[],
                            engine=inst.engine,
                            sync_info=_mybir.SyncInfo(on_wait=[w], on_update=[]),
                        )
                        insts.insert(i + j, nop)
                    i += len(waits) - 1
                i += 1


def _build(ids):
    f32 = mybir.dt.float32
    P = 2 * C  # q on partitions 0-63, k on 64-127
    nc = bass.Bass()
    # q and k stacked: the (tensor, channel) dims merge into one uniform
    # 128-row stride, so a single DMA per sample feeds all 16 SDMA ports.
    fqk = nc.dram_tensor("fqk", [P, H, W], f32, kind="ExternalInput")
    # [I64; -I64] so (q_hat - k_hat) falls out of one K=128 matmul
    wdiff = nc.dram_tensor("wdiff", [P, C], f32, kind="ExternalInput")
    out = nc.dram_tensor("out", [1, 1], f32, kind="ExternalOutput")

    with tile.TileContext(nc) as tc, ExitStack() as ctx:
        sb = ctx.enter_context(tc.tile_pool(name="sb", bufs=1))
        work = ctx.enter_context(tc.tile_pool(name="work", bufs=3))
        pn = ctx.enter_context(tc.tile_pool(name="pn", bufs=1, space="PSUM"))
        pbc = ctx.enter_context(tc.tile_pool(name="pbc", bufs=2, space="PSUM"))
        pd = ctx.enter_context(tc.tile_pool(name="pd", bufs=2, space="PSUM"))
        pf = ctx.enter_context(tc.tile_pool(name="pf", bufs=1, space="PSUM"))

        ones = sb.tile([P, 1], f32)
        nc.vector.memset(ones[:], 1.0)
        ones_row = sb.tile([1, C], f32)
        nc.vector.memset(ones_row[:], 1.0)
        wd = sb.tile([P, C], f32)
        nc.sync.dma_start(out=wd[:], in_=wdiff[:])
        # PE warmup so later matmuls don't pay a fresh DVE-clock wait.
        warm = pf.tile([1, 1], f32, tag="warm")
        nc.tensor.matmul(
            out=warm[:], lhsT=ones[:], rhs=ones[:], start=True, stop=True
        )

        qkraw = sb.tile([P, NUM_S, 9], f32)
        # Gather 3x3 windows: ONE strided DMA per sample covering q and k
        # (12B contiguous runs x 3 rows x 128 stacked channels).  The
        # bottleneck is descriptor generation (~4 ns/descriptor per ring),
        # so spread samples over all three generators: SP and ACT HWDGE
        # rings plus the gpsimd SWDGE ring (a bit slower per descriptor).
        qeng = [
            nc.sync, nc.scalar, nc.gpsimd, nc.sync,
            nc.scalar, nc.sync, nc.scalar, nc.gpsimd,
        ]
        for s, (h, w) in enumerate(ids):
            qeng[s % 8].dma_start(
                out=qkraw[:, s, :], in_=fqk[:, h : h + 3, w : w + 3]
            )

        # Process samples in groups so compute streams behind the gathers.
        GS = 32  # samples per group
        GC = GS * 9  # 288 columns (matmul moving-free <= 512)
        NG = NUM_S // GS
        d = sb.tile([P, NUM_S, 9], f32)
        d2 = sb.tile([P, NUM_S, 9], f32)
        df_ = d[:].rearrange("p s n -> p (s n)")
        d2f = d2[:].rearrange("p s n -> p (s n)")
        # q norms in cols [0, NCOL), k norms in cols [NCOL, 2*NCOL): engine
        # writes must stay at partition base 0
        norm = sb.tile([1, 2 * NCOL], f32)
        rinv = sb.tile([1, 2 * NCOL], f32)
        acc = sb.tile([C, NG], f32)

        for g in range(NG):
            ss = slice(g * GS, (g + 1) * GS)
            sl = slice(g * GC, (g + 1) * GC)
            slk = slice(NCOL + g * GC, NCOL + (g + 1) * GC)
            # d = window - center (center column j=4 becomes exactly 0)
            nc.vector.tensor_tensor(
                out=d[:, ss, :],
                in0=qkraw[:, ss, :],
                in1=qkraw[:, ss, 4:5].to_broadcast([P, GS, 9]),
                op=mybir.AluOpType.subtract,
            )
            nc.scalar.square(out=d2[:, ss, :], in_=d[:, ss, :])
            # norm2[col] = sum_c d2[c, col], q and k halves separately
            n2q = pn.tile([1, GC], f32, tag="n2q")
            n2k = pn.tile([1, GC], f32, tag="n2k")
            nc.tensor.matmul(
                out=n2q[:], lhsT=ones[0:C, :], rhs=d2f[0:C, sl],
                start=True, stop=True,
            )
            nc.tensor.matmul(
                out=n2k[:], lhsT=ones[C:P, :], rhs=d2f[C:P, sl],
                start=True, stop=True,
            )
            nc.scalar.sqrt(out=norm[:, sl], in_=n2q[:])
            nc.scalar.sqrt(out=norm[:, slk], in_=n2k[:])
            # rinv = 1/(sqrt(norm2)+eps); center cols give d*(1/eps) = 0
            nc.vector.tensor_scalar_add(
                out=norm[:, sl], in0=norm[:, sl], scalar1=EPS
            )
            nc.vector.tensor_scalar_add(
                out=norm[:, slk], in0=norm[:, slk], scalar1=EPS
            )
            nc.vector.reciprocal(out=rinv[:, sl], in_=norm[:, sl])
            nc.vector.reciprocal(out=rinv[:, slk], in_=norm[:, slk])
            # two K=1 matmuls broadcast rinv_q/rinv_k onto partition
            # quadrants 0 and 64 of one PSUM tile
            bc = pbc.tile([P, GC], f32)
            nc.tensor.matmul(
                out=bc[0:C, :], lhsT=ones_row[:], rhs=rinv[:, sl],
                start=True, stop=True,
            )
            nc.tensor.matmul(
                out=bc[C:P, :], lhsT=ones_row[:], rhs=rinv[:, slk],
                start=True, stop=True,
            )
            qkh = work.tile([P, GC], f32, tag="qkh")
            nc.vector.tensor_tensor(
                out=qkh[:], in0=df_[:, sl], in1=bc[:], op=mybir.AluOpType.mult
            )
            # q_hat - k_hat across the partition halves via [I; -I] matmul
            dif = pd.tile([C, GC], f32, tag="dif")
            nc.tensor.matmul(
                out=dif[:], lhsT=wd[:], rhs=qkh[:], start=True, stop=True
            )
            nc.vector.tensor_reduce(
                out=acc[:, g : g + 1],
                in_=dif[:],
                axis=mybir.AxisListType.X,
                op=mybir.AluOpType.add,
                apply_absolute_value=True,
            )

        accs = sb.tile([C, 1], f32)
        nc.vector.tensor_reduce(
            out=accs[:], in_=acc[:], axis=mybir.AxisListType.X, op=mybir.AluOpType.add
        )
        pfin = pf.tile([1, 1], f32, tag="fin")
        nc.tensor.matmul(
            out=pfin[:], lhsT=accs[:], rhs=ones[0:C, :], start=True, stop=True
        )
        res = sb.tile([1, 1], f32)
        nc.scalar.copy(out=res[:], in_=pfin[:])
        nc.gpsimd.dma_start(out=out[:], in_=res[:])

    _split_multi_waits(nc)
    return nc


def kernel(feat_q, feat_k, sample_ids, *, trace=False, trace_cores=None):
    global LAST_RESULTS
    feat_q = np.ascontiguousarray(np.asarray(feat_q), dtype=np.float32)
    feat_k = np.ascontiguousarray(np.asarray(feat_k), dtype=np.float32)
    ids = np.asarray(sample_ids)
    ids_key = tuple(map(tuple, ids.astype(np.int64).tolist()))
    if ids_key not in _cache:
        _cache[ids_key] = _build(ids_key)
    nc = _cache[ids_key]

    eye = np.eye(C, dtype=np.float32)
    wd = np.concatenate([eye, -eye], axis=0)  # [128, 64]
    in_maps = [
        {
            "fqk": np.concatenate([feat_q[b], feat_k[b]], axis=0),
            "wdiff": wd,
        }
        for b in range(N_CORES)
    ]
    results = run_bass_kernel_spmd(
        nc,
        in_maps,
        core_ids=list(range(N_CORES)),
        trace=trace,
        trace_cores=trace_cores,
    )
    LAST_RESULTS = results
    total = np.float64(0.0)
    for r in results.results:
        total += np.float64(r["out"][0, 0])
    loss = total / (B * C * 8 * NUM_S)
    return np.asarray(loss, dtype=np.float32)



# revision 10
# speedup vs baseline: 5.5867x; 5.5867x over previous
"""CCPL contrastive-loss kernel for Trainium2 (8 NeuronCores).

Strategy: the loss only touches 256 sampled 3x3 neighborhoods of
feat_q/feat_k (~4.7 MB of each 512 MiB tensor), so the kernel never
streams the full tensors.  Work is data-parallel over the batch dim:
core b receives feat_q[b] / feat_k[b] re-laid-out channel-last
([H*W, 128] with q on channels 0-63, k on 64-127), so each sampled
pixel's 128 channels are one contiguous 512 B run in HBM.  The 2304
window positions (256 samples x 9) are gathered by two
indirect_dma_start instructions (SWDGE descriptor generation:
~1 us fixed + 0.34 ns/descriptor) using a [128, 18] int32 row-index
table that ships as *data*, so the program never recompiles when
sample_ids change.  Samples land on SBUF partitions (s%128), channels
on the free axis; per-(sample, position, tensor) L2 norms are free-axis
block reductions on DVE, the normalize/subtract pipeline is split
across Pool/ACT/DVE, and the final cross-partition sum is one PE
matmul.  The host sums the 8 per-core partials and divides by the
element count.
"""

import os
import sys
from contextlib import ExitStack

import numpy as np

sys.path.insert(0, "/opt/trn_rl_repo")

import concourse.bass as bass
import concourse.tile as tile
from concourse import mybir
from concourse.bass_utils import run_bass_kernel_spmd


def _install_ntff_hook():
    """Provide antenv.axon_hooks when the agent image lacks it.

    concourse's axon trace path imports antenv.axon_hooks to fetch the
    NTFF profile hook; this image's antenv has no such submodule.  The
    hook implementation ships in trn_agent_boot.trn_boot, so wire it up
    against the axon PJRT .so directly.
    """
    try:
        from antenv.axon_hooks import get_axon_ntff_profile_hook  # noqa: F401

        return
    except ImportError:
        pass
    import types

    hook = None
    try:
        from trn_agent_boot.trn_boot import _ntff_profile_via_ctypes

        so = "/opt/axon/libaxon_pjrt.so"
        if os.path.exists(so):
            hook = _ntff_profile_via_ctypes(so)
    except Exception:
        hook = None
    mod = types.ModuleType("antenv.axon_hooks")
    _state = {"hook": hook}
    mod.get_axon_ntff_profile_hook = lambda: _state["hook"]
    mod.set_axon_ntff_profile_hook = lambda h: _state.update(hook=h)
    import antenv

    sys.modules["antenv.axon_hooks"] = mod
    antenv.axon_hooks = mod


_install_ntff_hook()

B, C, H, W = 8, 64, 512, 512
NUM_S = 256
P = 2 * C  # q on channels 0-63, k on 64-127 of the channel-last layout
NSLOT = 2  # 256 samples -> 2 slots of 128 (sample s = slot*128 + partition)
N_CORES = 8

_cache: dict = {}
LAST_RESULTS = None  # BassKernelResults of the most recent run (for test.py)


def _split_multi_waits(nc):
    """Walrus build here embeds at most ONE sync wait per instruction.

    Tile emits instructions (notably the kernel-tail Drain) carrying many
    sem waits.  Hoist all but the last wait of any such instruction onto
    single-wait NOPs inserted immediately before it on the same queue —
    the queue stalls on each NOP in turn, preserving semantics.
    """
    from concourse import mybir as _mybir

    for f in nc.m.functions:
        for blk in f.blocks:
            insts = blk.instructions
            i = 0
            while i < len(insts):
                inst = insts[i]
                si = inst.sync_info
                if si is not None and si.on_wait and len(si.on_wait) > 1:
                    waits = list(si.on_wait)
                    si.on_wait = waits[-1:]
                    for j, w in enumerate(waits[:-1]):
                        nop = _mybir.InstNoOp(
                            name=nc.get_next_instruction_name(),
                            ins=[],
                            outs=[],
                            engine=inst.engine,
                            sync_info=_mybir.SyncInfo(on_wait=[w], on_update=[]),
                        )
                        insts.insert(i + j, nop)
                    i += len(waits) - 1
                i += 1


def _build(split_waits=True):
    f32 = mybir.dt.float32
    i32 = mybir.dt.int32
    sub = mybir.AluOpType.subtract
    mul = mybir.AluOpType.mult
    add = mybir.AluOpType.add
    nc = bass.Bass()

    # channel-last stacked features: row (h*512 + w) holds the 128 q|k
    # channels of pixel (h, w) as one contiguous 512 B run.
    fqkT = nc.dram_tensor("fqkT", [H * W, P], f32, kind="ExternalInput")
    # idx[p, t*3 + r] = (h+r)*512 + w for sample s = t*128 + p.  The HW
    # indirect-DMA ucode consumes exactly ONE offset per destination
    # partition (multi-offset APs silently misgather), so the window
    # gather is 6 calls of shape offsets=[128,1] -> out=[128, 3*128]
    # (rows w..w+2 are contiguous channel-last, 1536 B per partition).
    idxT = nc.dram_tensor("idx", [128, NSLOT * 3], i32, kind="ExternalInput")
    out = nc.dram_tensor("out", [NSLOT, 1], f32, kind="ExternalOutput")

    with tile.TileContext(nc) as tc, ExitStack() as ctx:
        sb = ctx.enter_context(tc.tile_pool(name="sb", bufs=1))
        pf = ctx.enter_context(tc.tile_pool(name="pf", bufs=1, space="PSUM"))

        ones = sb.tile([128, 1], f32)
        nc.vector.memset(ones[:], 1.0)
        idx = sb.tile([128, NSLOT * 3], i32)
        nc.sync.dma_start(out=idx[:], in_=idxT[:])
        # engine warmups: PE clock + ACT sqrt-table load happen off the
        # critical path while the index table streams in.
        warm = pf.tile([1, 1], f32, tag="warm")
        nc.tensor.matmul(out=warm[:], lhsT=ones[:], rhs=ones[:], start=True, stop=True)
        actw = sb.tile([128, 1], f32)
        nc.scalar.sqrt(out=actw[:], in_=ones[:])
        tiny = sb.tile([128, 1], f32)
        nc.vector.memset(tiny[:], 1e-14)

        qk = sb.tile([128, NSLOT, 9, P], f32)  # gathered windows
        d = sb.tile([128, NSLOT, 9, P], f32)  # window - center
        d2 = sb.tile([128, NSLOT, 9, P], f32)
        xh = sb.tile([128, NSLOT, 9, P], f32)  # normalized (q_hat | k_hat)
        n2 = sb.tile([128, NSLOT, 18], f32)  # block B = j*2 + (0:q, 1:k)
        nrm = sb.tile([128, NSLOT, 18], f32)
        rinv = sb.tile([128, NSLOT, 18], f32)
        dif = sb.tile([128, NSLOT, 9, C], f32)
        acc = sb.tile([128, NSLOT], f32)

        # 6 SWDGE gathers (slot-major so slot 0 lands first): each brings
        # one window row (3 positions x 128 ch) for 128 samples.
        qkr = qk[:].rearrange("p t (r dw) c -> p t r (dw c)", r=3)
        for t in range(NSLOT):
            for r in range(3):
                nc.gpsimd.indirect_dma_start(
                    out=qkr[:, t, r],
                    out_offset=None,
                    in_=fqkT[:],
                    in_offset=bass.IndirectOffsetOnAxis(
                        ap=idx[:, t * 3 + r : t * 3 + r + 1], axis=0
                    ),
                )

        for t in range(NSLOT):
            # center-subtract: slot 0 on DVE (Pool is still generating
            # descriptors for slot 1), slot 1 on Pool (DVE is mid-pipeline).
            ctr = qk[:, t, 4:5, :].to_broadcast([128, 9, P])
            eng = nc.vector if t == 0 else nc.gpsimd
            eng.tensor_tensor(out=d[:, t], in0=qk[:, t], in1=ctr, op=sub)
            nc.scalar.square(out=d2[:, t], in_=d[:, t])
            # norm^2 per (position, tensor) block of 64 channels
            d2b = d2[:, t].rearrange("p j (b c) -> p (j b) c", b=2)
            nc.vector.tensor_reduce(
                out=n2[:, t], in_=d2b, axis=mybir.AxisListType.X, op=add
            )
            # rinv = 1/sqrt(norm2 + tiny); center block norm2=0 -> d=0 -> 0
            nc.scalar.activation(
                out=nrm[:, t], in_=n2[:, t],
                func=mybir.ActivationFunctionType.Sqrt, bias=tiny[:],
            )
            nc.vector.reciprocal(out=rinv[:, t], in_=nrm[:, t])
            dq = d[:, t].rearrange("p j (b c) -> p j b c", b=2)
            xq = xh[:, t].rearrange("p j (b c) -> p j b c", b=2)
            rv = rinv[:, t].rearrange("p (j b) -> p j b", b=2)
            # normalize: q half on Pool, k half on DVE
            nc.gpsimd.tensor_tensor(
                out=xq[:, :, 0],
                in0=dq[:, :, 0],
                in1=rv[:, :, 0:1].to_broadcast([128, 9, C]),
                op=mul,
            )
            nc.vector.tensor_tensor(
                out=xq[:, :, 1],
                in0=dq[:, :, 1],
                in1=rv[:, :, 1:2].to_broadcast([128, 9, C]),
                op=mul,
            )
            nc.vector.tensor_tensor(
                out=dif[:, t], in0=xq[:, :, 0], in1=xq[:, :, 1], op=sub
            )
            nc.vector.tensor_reduce(
                out=acc[:, t : t + 1],
                in_=dif[:, t].rearrange("p j c -> p (j c)"),
                axis=mybir.AxisListType.X,
                op=add,
                apply_absolute_value=True,
            )

        # cross-partition sum: out[t] = sum_p acc[p, t]
        pfin = pf.tile([NSLOT, 1], f32, tag="fin")
        nc.tensor.matmul(out=pfin[:], lhsT=acc[:], rhs=ones[:], start=True, stop=True)
        res = sb.tile([NSLOT, 1], f32)
        nc.scalar.copy(out=res[:], in_=pfin[:])
        nc.sync.dma_start(out=out[:], in_=res[:])

    if split_waits:
        _split_multi_waits(nc)
    return nc


def kernel(feat_q, feat_k, sample_ids, *, trace=False, trace_cores=None):
    global LAST_RESULTS
    feat_q = np.asarray(feat_q, dtype=np.float32)
    feat_k = np.asarray(feat_k, dtype=np.float32)
    ids = np.asarray(sample_ids).astype(np.int64)

    if "prog" not in _cache:
        _cache["prog"] = _build()
    nc = _cache["prog"]

    # idx[p, t*3 + r] = flat position of window row r for sample t*128 + p
    hs, ws = ids[:, 0], ids[:, 1]
    r = np.arange(3)
    rowpos = (hs[:, None] + r[None, :]) * W + ws[:, None]  # [256, 3]
    idx = np.ascontiguousarray(
        rowpos.reshape(NSLOT, 128, 3).transpose(1, 0, 2).reshape(128, NSLOT * 3)
    ).astype(np.int32)

    in_maps = []
    for b in range(N_CORES):
        fqk = np.concatenate([feat_q[b], feat_k[b]], axis=0)  # [128, H, W]
        fqkT = np.ascontiguousarray(fqk.transpose(1, 2, 0)).reshape(H * W, P)
        in_maps.append({"fqkT": fqkT, "idx": idx})

    results = run_bass_kernel_spmd(
        nc,
        in_maps,
        core_ids=list(range(N_CORES)),
        trace=trace,
        trace_cores=trace_cores,
    )
    LAST_RESULTS = results
    total = np.float64(0.0)
    for r in results.results:
        total += np.float64(r["out"].sum())
    loss = total / (B * C * 8 * NUM_S)
    return np.asarray(loss, dtype=np.float32)
